# revision 1
# baseline (speedup 1.0000x reference)
"""Trainium2 Bass kernel v2 for BasicGNNEncoder (gnn_message_passing).

Structure (per core, dst-sharded, SPMD-uniform):
  - fp16 everywhere on the edge/GEMM path (fp32 PSUM accumulation).
  - h state lives hid-major in SBUF: hT [128, npadc] fp16.
  - Per layer, chunk-major (512 dst cols at a time):
      * dma_gather (fp16 rows, 4 SWDGE queues round-robin) of src states
      * one-hot segment-sum matmuls accumulate ALL of a chunk's tiles
        (4 src-groups) directly into one PSUM bank
      * GRU update in transposed land; edge GEMM folded into GRU input
        weights host-side (Wfold = wih @ ew, bfold = wih @ eb)
      * PE-transpose store to node-major DRAM for the next AllGather
  - AllGather full h (fp16, node-major) between layers.
"""

import hashlib
import sys

import numpy as np

for _p in ("/opt/trn_rl_repo",):
    if _p not in sys.path:
        sys.path.insert(0, _p)

import concourse.bass as bass  # noqa: E402
import concourse.bacc as bacc  # noqa: E402
import concourse.mybir as mybir  # noqa: E402
import concourse.tile as tile  # noqa: E402

P = 128
BLKW = 64
CHUNKW = 512
F32 = mybir.dt.float32
F16 = mybir.dt.float16
I16 = mybir.dt.int16

NQ = 4           # SWDGE queues for dma_gather
BATCH_TILES = 8  # tiles per dma_gather call (<= 8 to stay within one group run)


def _cdiv(a, b):
    return (a + b - 1) // b


# ===========================================================================
# Host-side preprocessing: SPMD-uniform, chunk-major edge schedule
# ===========================================================================
class Schedule:
    pass


def _preprocess(edge_index, n_nodes, n_cores):
    src = np.asarray(edge_index[0], dtype=np.int64)
    dst = np.asarray(edge_index[1], dtype=np.int64)

    s = Schedule()
    s.n_nodes = n_nodes
    s.n_cores = n_cores
    s.shard = _cdiv(n_nodes, n_cores)
    s.npadc = _cdiv(s.shard, P) * P
    s.npad_all = s.npadc * n_cores
    s.n_groups = 4
    s.gch = _cdiv(s.npad_all, s.n_groups)
    assert s.gch <= 32768, s.gch
    s.n_blocks = _cdiv(s.shard, BLKW)
    s.chunks = []
    c0 = 0
    while c0 < s.npadc:
        w = min(CHUNKW, s.npadc - c0)
        s.chunks.append((c0, w))
        c0 += w

    owner = src // s.shard
    row = owner * s.npadc + (src % s.shard)
    grp = row // s.gch
    core = dst // s.shard
    d = dst % s.shard
    blk = d // BLKW

    E = np.zeros((n_cores, s.n_groups, s.n_blocks), np.int64)
    np.add.at(E, (core, grp, blk), 1)
    T = np.maximum(1, _cdiv(E.max(axis=0), P)).astype(np.int64)  # [g, b]
    s.T = T

    # chunk-major tile stream: for chunk: for g: for b in chunk: T[g,b] tiles
    tile_meta = []  # (g, b, ci, slot, start_of_chunk, stop_of_chunk)
    for ci, (c0, w) in enumerate(s.chunks):
        b_lo = c0 // BLKW
        b_hi = b_lo + w // BLKW
        n_in_chunk = int(T[:, b_lo:b_hi].sum())
        j = 0
        for g in range(s.n_groups):
            for b in range(b_lo, b_hi):
                for _k in range(int(T[g, b])):
                    tile_meta.append(
                        (g, b, ci, b - b_lo, j == 0, j == n_in_chunk - 1)
                    )
                    j += 1
    s.n_tiles = len(tile_meta)
    s.tile_meta = tile_meta

    # gather calls: consecutive tiles sharing g, up to BATCH_TILES
    calls = []  # (t0, bt, g, queue)
    t0 = 0
    qi = 0
    while t0 < s.n_tiles:
        g = tile_meta[t0][0]
        bt = 1
        while (
            bt < BATCH_TILES
            and t0 + bt < s.n_tiles
            and tile_meta[t0 + bt][0] == g
        ):
            bt += 1
        calls.append((t0, bt, g, qi % NQ))
        qi += 1
        t0 += bt
    s.calls = calls

    # per-core idx (int16, group-relative) and dl (fp16 dst offset in block)
    order = np.lexsort((d, blk, grp, core))
    row_o = row[order]
    d_o = d[order]
    keys = ((core * s.n_groups + grp) * s.n_blocks + blk)[order]
    nk = n_cores * s.n_groups * s.n_blocks
    cnt = np.bincount(keys, minlength=nk)
    starts = np.concatenate([[0], np.cumsum(cnt)])

    tidx = {}
    for t, (g, b, ci, sl, st, sp) in enumerate(tile_meta):
        tidx.setdefault((g, b), []).append(t)

    idx_all = np.zeros((n_cores, s.n_tiles, P), np.int16)
    dl_all = np.full((n_cores, s.n_tiles, P), -1.0, np.float16)
    for c in range(n_cores):
        for g in range(s.n_groups):
            for b in range(s.n_blocks):
                k = (c * s.n_groups + g) * s.n_blocks + b
                lo, hi = starts[k], starts[k + 1]
                tl = tidx[(g, b)]
                assert hi - lo <= len(tl) * P
                for j, t in enumerate(tl):
                    e0 = lo + j * P
                    e1 = min(lo + (j + 1) * P, hi)
                    if e1 <= e0:
                        break
                    n = e1 - e0
                    idx_all[c, t, :n] = (row_o[e0:e1] - g * s.gch).astype(
                        np.int16
                    )
                    dl_all[c, t, :n] = (d_o[e0:e1] - b * BLKW).astype(
                        np.float16
                    )

    # idx stream in dma_gather layout: [128, 8*n_tiles] int16
    s.idx_arrs = []
    s.dl_arrs = []
    for c in range(n_cores):
        flat = idx_all[c].reshape(-1)                   # tile stream
        cols = flat.reshape(-1, 16).T                   # [16, 8*n_tiles]
        s.idx_arrs.append(np.ascontiguousarray(np.tile(cols, (8, 1))))
        s.dl_arrs.append(
            np.ascontiguousarray(dl_all[c].transpose(1, 0))  # [128, n_tiles]
        )

    # degree per core (padded cols)
    deg = np.zeros((n_cores, s.npadc), np.float16)
    cnt_d = np.bincount(dst, minlength=n_nodes).astype(np.float16)
    for c in range(n_cores):
        lo = c * s.shard
        hi = min(n_nodes, (c + 1) * s.shard)
        deg[c, : hi - lo] = cnt_d[lo:hi]
    ones = np.ones((n_cores, 1, s.npadc), np.float16)
    s.deg = np.concatenate([deg.reshape(n_cores, 1, s.npadc), ones], axis=1)
    return s


# ===========================================================================
# Program builder
# ===========================================================================
def _build_program(s, feat, hid, n_layers, debug=False):
    assert hid == P and feat % P == 0
    kf = feat // P
    nc = bacc.Bacc(
        "TRN2",
        target_bir_lowering=False,
        debug=debug,
        num_devices=s.n_cores,
        num_swdge_queues=NQ,
    )

    # ---- I/O ----
    xT = nc.dram_tensor("xT", [feat, s.npadc], F16, kind="ExternalInput")
    degt = nc.dram_tensor("deg", [2, s.npadc], F16, kind="ExternalInput")
    dstloc = nc.dram_tensor("dstloc", [P, s.n_tiles], F16, kind="ExternalInput")
    idx_t = nc.dram_tensor(
        "idx", [P, 8 * s.n_tiles], I16, kind="ExternalInput"
    )
    iotat = nc.dram_tensor(
        "iota64", [P, BATCH_TILES * BLKW], F16, kind="ExternalInput"
    )
    ident = nc.dram_tensor("ident", [P, P], F16, kind="ExternalInput")
    wpT = nc.dram_tensor("wpT", [feat, P], F16, kind="ExternalInput")
    bp = nc.dram_tensor("bp", [P, 1], F32, kind="ExternalInput")
    # folded GRU input weights: [L, hid, 3*hid] (r, z, n)
    wfT = nc.dram_tensor("wfT", [n_layers, P, 3 * P], F16, kind="ExternalInput")
    whhT = nc.dram_tensor(
        "whhT", [n_layers, P, 3 * P], F16, kind="ExternalInput"
    )
    # per-gate bias rows, applied as K=2 matmul against [deg; ones]:
    #   region g in {r,z,n,hn}: lhsT = bf2[:, g*P:(g+1)*P] = [deg_coeff; ones_coeff]
    #   r: [bfold_r; 0]  z: [bfold_z; 0]  n: [bfold_n; bih_n]  hn: [0; bhh_n]
    bf2 = nc.dram_tensor("bf2", [n_layers, 2, 4 * P], F16, kind="ExternalInput")
    # ACT biases for r/z sigmoid: [L, hid, 2]
    gbact = nc.dram_tensor("gbact", [n_layers, P, 2], F32, kind="ExternalInput")
    h_out = nc.dram_tensor("h_out", [s.npadc, P], F32, kind="ExternalOutput")

    h_own = [
        nc.dram_tensor(f"h_own{l}", [s.npadc, P], F16) for l in range(n_layers)
    ]
    h_full = [
        nc.dram_tensor(
            f"h_full{l}", [s.npad_all, P], F16, addr_space="Shared"
        )
        for l in range(n_layers)
    ]
    rg = [list(range(s.n_cores))]

    from contextlib import ExitStack

    with tile.TileContext(nc) as tc, ExitStack() as ctx:
        consts = ctx.enter_context(tc.tile_pool(name="consts", bufs=1))
        sb_in = ctx.enter_context(tc.tile_pool(name="sb_in", bufs=3))
        sb_stg = ctx.enter_context(tc.tile_pool(name="sb_stg", bufs=3))
        sb_big = ctx.enter_context(tc.tile_pool(name="sb_big", bufs=1))
        sb_gru = ctx.enter_context(tc.tile_pool(name="sb_gru", bufs=2))
        sb_st = ctx.enter_context(tc.tile_pool(name="sb_st", bufs=3))
        psum = ctx.enter_context(
            tc.tile_pool(name="psum", bufs=2, space="PSUM")
        )

        # ---- constants ----
        iota_sb = consts.tile(
            [P, BATCH_TILES * BLKW], F16, tag="iota", name="iota_sb"
        )
        nc.sync.dma_start(out=iota_sb[:], in_=iotat[:, :])
        iden_sb = consts.tile([P, P], F16, tag="iden", name="iden_sb")
        nc.sync.dma_start(out=iden_sb[:], in_=ident[:, :])
        wp_sb = [
            consts.tile([P, P], F16, tag=f"wp{k}", name=f"wp_sb{k}")
            for k in range(kf)
        ]
        for k in range(kf):
            nc.sync.dma_start(out=wp_sb[k][:], in_=wpT[k * P : (k + 1) * P, :])
        bp_sb = consts.tile([P, 1], F32, tag="bp", name="bp_sb")
        nc.sync.dma_start(out=bp_sb[:], in_=bp[:, :])
        wf_sb = [
            consts.tile([P, 3 * P], F16, tag=f"wf{l}", name=f"wf_sb{l}")
            for l in range(n_layers)
        ]
        whh_sb = [
            consts.tile([P, 3 * P], F16, tag=f"whh{l}", name=f"whh_sb{l}")
            for l in range(n_layers)
        ]
        bf2_sb = [
            consts.tile([2, 4 * P], F16, tag=f"bf2{l}", name=f"bf2_sb{l}")
            for l in range(n_layers)
        ]
        gba_sb = [
            consts.tile([P, 2], F32, tag=f"gba{l}", name=f"gba_sb{l}")
            for l in range(n_layers)
        ]
        for l in range(n_layers):
            nc.sync.dma_start(out=wf_sb[l][:], in_=wfT[l])
            nc.sync.dma_start(out=whh_sb[l][:], in_=whhT[l])
            nc.sync.dma_start(out=bf2_sb[l][:], in_=bf2[l])
            nc.sync.dma_start(out=gba_sb[l][:], in_=gbact[l])
        # [deg; ones] rows for the K=2 bias matmuls
        dgo = consts.tile([2, s.npadc], F16, tag="dgo", name="dgo")
        nc.sync.dma_start(out=dgo[:, :], in_=degt[:, :])
        # resident idx + dl streams (shared by both layers)
        idx_sb = consts.tile([P, 8 * s.n_tiles], I16, tag="idx", name="idx_sb")
        nc.sync.dma_start(out=idx_sb[:], in_=idx_t[:, :])
        dl_sb = consts.tile([P, s.n_tiles], F16, tag="dl", name="dl_sb")
        nc.sync.dma_start(out=dl_sb[:], in_=dstloc[:, :])

        # persistent transposed state
        hT = sb_big.tile([P, s.npadc], F16, tag="hT", name="hT")

        def transpose_store(dst_dram, c0, w, cast_dt):
            tp = psum.tile([P, CHUNKW], F16, tag="tr", name="tp")
            for j in range(w // P):
                nc.tensor.transpose(
                    out=tp[:, j * P : (j + 1) * P],
                    in_=hT[:, c0 + j * P : c0 + (j + 1) * P],
                    identity=iden_sb[:],
                )
            st = sb_st.tile([P, CHUNKW], cast_dt, tag="tst", name="tst")
            nc.scalar.copy(out=st[:, :w], in_=tp[:, :w])
            for j in range(w // P):
                nc.sync.dma_start(
                    out=dst_dram[c0 + j * P : c0 + (j + 1) * P, :],
                    in_=st[:, j * P : (j + 1) * P],
                )

        # ---- projection (x loaded in chunk-aligned half tiles) ----
        n_chunks = len(s.chunks)
        nc2 = _cdiv(n_chunks, 2)
        xsplits = []  # (col0, col1) per half
        for h in range(2):
            cis = list(range(h * nc2, min((h + 1) * nc2, n_chunks)))
            if not cis:
                continue
            x0 = s.chunks[cis[0]][0]
            x1 = s.chunks[cis[-1]][0] + s.chunks[cis[-1]][1]
            xsplits.append((x0, x1))
        xw_max = max(x1 - x0 for x0, x1 in xsplits)
        xa_t = [
            [
                consts.tile([P, xw_max], F16, tag=f"xa{k}h{h}", name="xa")
                for k in range(kf)
            ]
            for h in range(len(xsplits))
        ]
        for h, (x0, x1) in enumerate(xsplits):
            for k in range(kf):
                nc.sync.dma_start(
                    out=xa_t[h][k][:, : x1 - x0],
                    in_=xT[k * P : (k + 1) * P, x0:x1],
                )
        for ci, (c0, w) in enumerate(s.chunks):
            ps = psum.tile([P, CHUNKW], F32, tag="seg", name="ps_seg")
            h = min(ci // nc2, len(xsplits) - 1)
            x0 = xsplits[h][0]
            for k in range(kf):
                nc.tensor.matmul(
                    out=ps[:, :w],
                    lhsT=wp_sb[k][:],
                    rhs=xa_t[h][k][:, c0 - x0 : c0 - x0 + w],
                    start=(k == 0),
                    stop=(k == kf - 1),
                )
            nc.scalar.activation(
                out=hT[:, c0 : c0 + w],
                in_=ps[:, :w],
                func=mybir.ActivationFunctionType.Relu,
                bias=bp_sb[:, 0:1],
            )
            transpose_store(h_own[0], c0, w, F16)

        nc.gpsimd.collective_compute(
            "AllGather",
            mybir.AluOpType.bypass,
            replica_groups=rg,
            ins=[h_own[0][:, :]],
            outs=[h_full[0][:, :]],
        )

        # ---- layers ----
        # map tile -> call
        call_of_tile = {}
        for k, (t0, bt, g, q) in enumerate(s.calls):
            for j in range(bt):
                call_of_tile[t0 + j] = (k, j)

        for l in range(n_layers):
            hf = h_full[l]
            stg_tiles = {}      # call idx -> stg tile
            oh_tiles = {}       # call idx -> one-hot tile
            ps_seg = None
            ps_ci = -1

            def issue_call(k):
                t0, bt, g, q = s.calls[k]
                rows_g = min(s.gch, s.npad_all - g * s.gch)
                stg = sb_stg.tile(
                    [P, BATCH_TILES, P], F16, tag=f"stg{q}", name="stg"
                )
                nc.gpsimd.dma_gather(
                    stg[:, :bt, :],
                    hf[g * s.gch : g * s.gch + rows_g, :],
                    idx_sb[:, 8 * t0 : 8 * (t0 + bt)],
                    num_idxs=P * bt,
                    num_idxs_reg=P * bt,
                    elem_size=P,
                    queue_num=q,
                )
                oh = sb_in.tile(
                    [P, BATCH_TILES * BLKW], F16, tag="oh", name="oh"
                )
                nc.vector.tensor_tensor(
                    out=oh[:, : bt * BLKW].rearrange(
                        "p (t j) -> p t j", j=BLKW
                    ),
                    in0=dl_sb[:, t0 : t0 + bt, None].to_broadcast(
                        [P, bt, BLKW]
                    ),
                    in1=iota_sb[:, : bt * BLKW].rearrange(
                        "p (t j) -> p t j", j=BLKW
                    ),
                    op=mybir.AluOpType.is_equal,
                )
                stg_tiles[k] = stg
                oh_tiles[k] = oh

            for t, (g, b, ci, sl, st_, sp_) in enumerate(s.tile_meta):
                k, j = call_of_tile[t]
                if k not in stg_tiles:
                    issue_call(k)
                if st_:
                    assert ps_seg is None
                    ps_seg = psum.tile([P, CHUNKW], F32, tag="seg", name="ps_seg")
                    ps_ci = ci
                assert ps_ci == ci
                nc.tensor.matmul(
                    out=ps_seg[:, sl * BLKW : (sl + 1) * BLKW],
                    lhsT=stg_tiles[k][:, j, :],
                    rhs=oh_tiles[k][:, j * BLKW : (j + 1) * BLKW],
                    start=st_,
                    stop=sp_,
                    skip_group_check=True,
                )
                if sp_:
                    # chunk ci aggregation complete -> GRU update
                    c0, w = s.chunks[ci]
                    sl_c = slice(c0, c0 + w)
                    aggc = sb_gru.tile([P, CHUNKW], F16, tag="aggc", name="aggc")
                    nc.scalar.copy(out=aggc[:, :w], in_=ps_seg[:, :w])
                    ps_seg = None

                    def gate_ps(col, with_agg, with_h):
                        # Wfold_g @ agg + whh_g @ hT + bf2_g @ [deg; ones]
                        pg = psum.tile([P, CHUNKW], F32, tag="gate", name="pg")
                        if with_agg:
                            nc.tensor.matmul(
                                out=pg[:, :w],
                                lhsT=wf_sb[l][:, col * P : (col + 1) * P],
                                rhs=aggc[:, :w],
                                start=True,
                                stop=False,
                            )
                        if with_h:
                            hcol = 2 * P if col == 3 else col * P
                            nc.tensor.matmul(
                                out=pg[:, :w],
                                lhsT=whh_sb[l][:, hcol : hcol + P],
                                rhs=hT[:, sl_c],
                                start=not with_agg,
                                stop=False,
                            )
                        nc.tensor.matmul(
                            out=pg[:, :w],
                            lhsT=bf2_sb[l][:, col * P : (col + 1) * P],
                            rhs=dgo[:, sl_c],
                            start=False,
                            stop=True,
                        )
                        return pg

                    pr = gate_ps(0, True, True)
                    r = sb_gru.tile([P, CHUNKW], F16, tag="r", name="rt")
                    nc.scalar.activation(
                        out=r[:, :w], in_=pr[:, :w],
                        func=mybir.ActivationFunctionType.Sigmoid,
                        bias=gba_sb[l][:, 0:1],
                    )
                    pz = gate_ps(1, True, True)
                    z = sb_gru.tile([P, CHUNKW], F16, tag="z", name="zt")
                    nc.scalar.activation(
                        out=z[:, :w], in_=pz[:, :w],
                        func=mybir.ActivationFunctionType.Sigmoid,
                        bias=gba_sb[l][:, 1:2],
                    )
                    # pi = Wfold_n @ agg + [bfold_n; bih_n] @ [deg; ones]
                    pi = gate_ps(2, True, False)
                    # ph = whh_n @ hT + [0; bhh_n] @ [deg; ones]
                    ph = gate_ps(3, False, True)
                    # t1 = r * ph + pi ; n = tanh(t1)
                    t1 = sb_gru.tile([P, CHUNKW], F32, tag="t1", name="t1")
                    nc.vector.tensor_mul(out=t1[:, :w], in0=r[:, :w], in1=ph[:, :w])
                    nc.vector.tensor_add(out=t1[:, :w], in0=t1[:, :w], in1=pi[:, :w])
                    n_t = sb_gru.tile([P, CHUNKW], F16, tag="nt", name="n_t")
                    nc.scalar.activation(
                        out=n_t[:, :w], in_=t1[:, :w],
                        func=mybir.ActivationFunctionType.Tanh,
                    )
                    t3 = sb_gru.tile([P, CHUNKW], F16, tag="t3", name="t3")
                    nc.vector.tensor_sub(out=t3[:, :w], in0=hT[:, sl_c], in1=n_t[:, :w])
                    nc.vector.tensor_mul(out=t3[:, :w], in0=z[:, :w], in1=t3[:, :w])
                    nc.vector.tensor_add(out=hT[:, sl_c], in0=n_t[:, :w], in1=t3[:, :w])

                    if l < n_layers - 1:
                        transpose_store(h_own[l + 1], c0, w, F16)
                    else:
                        transpose_store(h_out, c0, w, F32)

            if l < n_layers - 1:
                nc.gpsimd.collective_compute(
                    "AllGather",
                    mybir.AluOpType.bypass,
                    replica_groups=rg,
                    ins=[h_own[l + 1][:, :]],
                    outs=[h_full[l + 1][:, :]],
                )

    nc.compile()
    return nc


# ===========================================================================
# Input packing
# ===========================================================================
def _make_in_maps(s, inputs, feat, hid, n_layers):
    nf = np.asarray(inputs["node_features"], np.float32)
    w_proj = np.asarray(inputs["w_proj"], np.float64)
    b_proj = np.asarray(inputs["b_proj"], np.float64)
    edge_w = np.asarray(inputs["edge_w"], np.float64)
    edge_b = np.asarray(inputs["edge_b"], np.float64)
    gru_wih = np.asarray(inputs["gru_wih"], np.float64)
    gru_whh = np.asarray(inputs["gru_whh"], np.float64)
    gru_bih = np.asarray(inputs["gru_bih"], np.float64)
    gru_bhh = np.asarray(inputs["gru_bhh"], np.float64)

    n_nodes = nf.shape[0]
    xT = np.zeros((feat, s.npad_all), np.float16)
    xTv = nf.T
    for c in range(s.n_cores):
        lo = c * s.shard
        hi = min(n_nodes, (c + 1) * s.shard)
        xT[:, c * s.npadc : c * s.npadc + hi - lo] = xTv[:, lo:hi]

    iota = np.tile(
        np.arange(BLKW, dtype=np.float16), BATCH_TILES
    )[None, :].repeat(P, 0)
    ident = np.eye(P, dtype=np.float16)
    wpT = np.ascontiguousarray(w_proj.T).astype(np.float16)
    bp = b_proj.reshape(P, 1).astype(np.float32)

    # folded weights: Wfold_g = wih_g @ ew ; bfold_g = wih_g @ eb
    ew = edge_w[:, 0]                      # [L, H, H]
    eb = edge_b[:, 0]                      # [L, H]
    wfT = np.zeros((n_layers, P, 3 * P), np.float16)
    whhT = np.zeros((n_layers, P, 3 * P), np.float16)
    bf2 = np.zeros((n_layers, 2, 4 * P), np.float16)
    gba = np.zeros((n_layers, P, 2), np.float32)
    for l in range(n_layers):
        wf = gru_wih[l] @ ew[l]            # [3H, H]
        bf = gru_wih[l] @ eb[l]            # [3H]
        wfT[l] = wf.T.astype(np.float16)   # [H, 3H]
        whhT[l] = gru_whh[l].T.astype(np.float16)
        bf2[l, 0, 0:P] = bf[0:P]
        bf2[l, 0, P : 2 * P] = bf[P : 2 * P]
        bf2[l, 0, 2 * P : 3 * P] = bf[2 * P : 3 * P]
        bf2[l, 1, 2 * P : 3 * P] = gru_bih[l, 2 * P : 3 * P]
        bf2[l, 1, 3 * P : 4 * P] = gru_bhh[l, 2 * P : 3 * P]
        gba[l, :, 0] = gru_bih[l, 0:P] + gru_bhh[l, 0:P]
        gba[l, :, 1] = gru_bih[l, P : 2 * P] + gru_bhh[l, P : 2 * P]

    in_maps = []
    for c in range(s.n_cores):
        m = {
            "xT": np.ascontiguousarray(xT[:, c * s.npadc : (c + 1) * s.npadc]),
            "deg": s.deg[c],
            "dstloc": s.dl_arrs[c],
            "idx": s.idx_arrs[c],
            "iota64": iota,
            "ident": ident,
            "wpT": wpT,
            "bp": bp,
            "wfT": wfT,
            "whhT": whhT,
            "bf2": bf2,
            "gbact": gba,
        }
        in_maps.append(m)
    return in_maps


# ===========================================================================
# Public entry point
# ===========================================================================
_CACHE = {}


def _get_compiled(edge_index, n_nodes, feat, hid, n_layers, n_cores=8):
    key = hashlib.sha1(
        np.ascontiguousarray(edge_index).tobytes()
        + np.int64([n_nodes, feat, hid, n_layers, n_cores, 2]).tobytes()
    ).hexdigest()
    if key not in _CACHE:
        s = _preprocess(edge_index, n_nodes, n_cores)
        nc = _build_program(s, feat, hid, n_layers, debug=False)
        _CACHE[key] = (s, nc)
    return _CACHE[key]


def run(inputs, trace=False, tmpdir=None):
    from concourse.bass_utils import run_bass_kernel_spmd

    nf = np.asarray(inputs["node_features"])
    edge_index = np.asarray(inputs["edge_index"])
    n_nodes, feat = nf.shape
    hid = np.asarray(inputs["w_proj"]).shape[0]
    n_layers = np.asarray(inputs["gru_wih"]).shape[0]
    s, nc = _get_compiled(edge_index, n_nodes, feat, hid, n_layers)
    in_maps = _make_in_maps(s, inputs, feat, hid, n_layers)
    res = run_bass_kernel_spmd(
        nc, in_maps, core_ids=list(range(s.n_cores)), trace=trace,
        tmpdir=tmpdir,
    )
    out = np.empty((n_nodes, hid), np.float32)
    for c in range(s.n_cores):
        lo = c * s.shard
        hi = min(n_nodes, (c + 1) * s.shard)
        out[lo:hi] = res.results[c]["h_out"][: hi - lo]
    return out, res


def kernel(**inputs) -> np.ndarray:
    out, _ = run(inputs, trace=False)
    return out


# ===========================================================================
# Small-scale CoreSim self-test
# ===========================================================================
def _np_reference(inputs, n_layers):
    nf = np.asarray(inputs["node_features"], np.float64)
    src, dst = np.asarray(inputs["edge_index"], np.int64)
    w_proj = np.asarray(inputs["w_proj"], np.float64)
    h = np.maximum(nf @ w_proj.T + np.asarray(inputs["b_proj"], np.float64), 0)
    n = nf.shape[0]

    def sig(x):
        return 1.0 / (1.0 + np.exp(-x))

    for l in range(n_layers):
        ew = np.asarray(inputs["edge_w"], np.float64)[l, 0]
        ebv = np.asarray(inputs["edge_b"], np.float64)[l, 0]
        agg = np.zeros_like(h)
        np.add.at(agg, dst, h[src])
        deg = np.bincount(dst, minlength=n).astype(np.float64)[:, None]
        agg = agg @ ew.T + deg * ebv
        wih = np.asarray(inputs["gru_wih"], np.float64)[l]
        whh = np.asarray(inputs["gru_whh"], np.float64)[l]
        bih = np.asarray(inputs["gru_bih"], np.float64)[l]
        bhh = np.asarray(inputs["gru_bhh"], np.float64)[l]
        gi = agg @ wih.T + bih
        gh = h @ whh.T + bhh
        H = h.shape[1]
        r = sig(gi[:, :H] + gh[:, :H])
        z = sig(gi[:, H : 2 * H] + gh[:, H : 2 * H])
        nn_ = np.tanh(gi[:, 2 * H :] + r * gh[:, 2 * H :])
        h = (1 - z) * nn_ + z * h
    return h


def _selftest(n_nodes=3000, n_edges=20000, feat=256, hid=128, n_layers=2):
    import os
    from concourse.bass_interp import MultiCoreSim

    rng = np.random.default_rng(0)
    sc = 0.05
    inputs = {
        "node_features": rng.standard_normal((n_nodes, feat)).astype(np.float32),
        "edge_index": rng.integers(0, n_nodes, (2, n_edges), dtype=np.int64).astype(np.int32),
        "edge_type": np.zeros(n_edges, np.int32),
        "w_proj": (rng.standard_normal((hid, feat)) * sc).astype(np.float32),
        "b_proj": (rng.standard_normal(hid) * sc).astype(np.float32),
        "edge_w": (rng.standard_normal((n_layers, 1, hid, hid)) * sc).astype(np.float32),
        "edge_b": (rng.standard_normal((n_layers, 1, hid)) * sc).astype(np.float32),
        "gru_wih": (rng.standard_normal((n_layers, 3 * hid, hid)) * sc).astype(np.float32),
        "gru_whh": (rng.standard_normal((n_layers, 3 * hid, hid)) * sc).astype(np.float32),
        "gru_bih": (rng.standard_normal((n_layers, 3 * hid)) * sc).astype(np.float32),
        "gru_bhh": (rng.standard_normal((n_layers, 3 * hid)) * sc).astype(np.float32),
    }
    edge_index = inputs["edge_index"]
    s = _preprocess(edge_index, n_nodes, 8)
    print(
        f"schedule: tiles={s.n_tiles} calls={len(s.calls)} "
        f"npadc={s.npadc} gch={s.gch} blocks={s.n_blocks}"
    )
    nc = _build_program(s, feat, hid, n_layers, debug=False)
    in_maps = _make_in_maps(s, inputs, feat, hid, n_layers)

    exp = _np_reference(inputs, n_layers)
    out = np.empty((n_nodes, hid), np.float32)
    if os.environ.get("SELFTEST_HW", "0") == "1":
        from concourse.bass_utils import run_bass_kernel_spmd

        res = run_bass_kernel_spmd(nc, in_maps, core_ids=list(range(8)))
        for c in range(8):
            lo = c * s.shard
            hi = min(n_nodes, (c + 1) * s.shard)
            out[lo:hi] = res.results[c]["h_out"][: hi - lo]
    else:
        sim = MultiCoreSim(nc, 8)
        for c in range(8):
            for k, v in in_maps[c].items():
                sim.cores[c].tensor(k)[:] = v
        sim.simulate()
        for c in range(8):
            lo = c * s.shard
            hi = min(n_nodes, (c + 1) * s.shard)
            out[lo:hi] = sim.cores[c].mem_tensor("h_out")[: hi - lo]
    err = np.abs(out - exp).max() / max(1e-12, np.abs(exp).max())
    print("selftest rel absmax err:", err)
    assert err < 3e-3, err
    print("SELFTEST PASSED")


if __name__ == "__main__":
    _selftest()

